# revision 58
# baseline (speedup 1.0000x reference)
"""Multi-head causal self-attention (B=2, T=2048, D=1024, H=16) on 8 trn2 cores.

Sharding: data-parallel over batch (cores 0-3 -> batch 0, 4-7 -> batch 1),
tensor-parallel over heads within each 4-core group (4 heads per core).
Wq/Wk/Wv column-sharded, Wo row-sharded; each core emits its partial output
projection and the host sums the 4 partials per batch (TP unshard).

Schedule (per core): the scalar engine's exp stream (~80us) and the tensor
engine (~120us incl. projections) are kept concurrently busy:
  - all projections (q/k/v/out) are dripped one work-item per attention
    step into the scores->exp->attV steady state; emission order defines
    each engine's queue order
  - scores quad-pairs use tile_position row packing (both heads of a pair
    concurrently, K=64 each); causal masking multiplies the exp tiles
    in-place on DVE; diagonal tiles skip fully-masked query columns in the
    scores matmul, the exp (strided AP), the mask, and attV
  - attV pends 2 steps behind the exp stream (6 for qb0, draining through
    (1,0)) with a ones-column in v producing softmax denominators on psum
    partition 0
  - normalize: denom rows DMA-packed across partitions, one small DVE
    reciprocal, unpacked to a partition-0 row, gpsimd partition_broadcast,
    then one DVE multiply per head; all off the exp critical path
  - PSUM: scores 2x2 + attV accum 2 + proj/outproj 2 = 8 banks; the tail
    outproj borrows the dead score banks and splits casts across the
    scalar+vector engines; output DMAs alternate sync/gpsimd queues
  - PE warmup matmuls run during the input DMAs (HAM un-throttle) and
    during the final normalize chain
"""

import sys
from collections import deque

for _p in ("/opt/trn_rl_repo", "/root/.axon_site/_ro/trn_rl_repo"):
    if _p not in sys.path:
        sys.path.append(_p)

import ml_dtypes
import numpy as np

import concourse.bass as bass
import concourse.mybir as mybir
import concourse.tile as tile
from concourse import bacc
from concourse.bass_utils import run_bass_kernel_spmd

F32 = mybir.dt.float32
BF16 = mybir.dt.bfloat16

B, T, D = 2, 2048, 1024
H, DH = 16, 64
HPC = 4          # heads per core
FPC = HPC * DH   # feature dims per core (256)
NKT = T // 128   # 16 key tiles / token tiles
NQB = T // 512   # 4 query blocks
VW = 128         # v slot width: 64 dims + ones col 64 + zero pad (FWL needs 128)

DIAG_SKIP = True
N_WARM_MM = 8

_CACHE = {}


def _build():
    nc = bacc.Bacc("TRN2", target_bir_lowering=False, debug=False, num_devices=8)

    xt_d = nc.dram_tensor("xt", [D, T], BF16, kind="ExternalInput").ap()
    wq_d = nc.dram_tensor("wq_p", [128, 2 * D], BF16, kind="ExternalInput").ap()
    wk_d = nc.dram_tensor("wk_p", [128, 2 * D], BF16, kind="ExternalInput").ap()
    wv_d = nc.dram_tensor("wv_t", [128, 8 * FPC], BF16, kind="ExternalInput").ap()
    wo_d = nc.dram_tensor("wo_t", [128, 2 * D], BF16, kind="ExternalInput").ap()
    masks_d = nc.dram_tensor("masks", [128, 4 * 1024], BF16, kind="ExternalInput").ap()
    out_d = nc.dram_tensor("po", [T, D], BF16, kind="ExternalOutput").ap()

    with tile.TileContext(nc) as tc:
        with (
            tc.tile_pool(name="wp", bufs=1) as wp,
            tc.tile_pool(name="xp", bufs=1) as xp,
            tc.tile_pool(name="qk", bufs=1) as qk,
            tc.tile_pool(name="vp", bufs=1) as vp,
            tc.tile_pool(name="at", bufs=1) as at,
            tc.tile_pool(name="ep", bufs=1) as ep,
            tc.tile_pool(name="ob", bufs=1) as ob,
            tc.tile_pool(name="nr", bufs=2) as nr,
            tc.tile_pool(name="op", bufs=3) as op,
            tc.tile_pool(name="pss", bufs=1, space="PSUM") as pss,
            tc.tile_pool(name="pso", bufs=1, space="PSUM") as pso,
            tc.tile_pool(name="psp", bufs=1, space="PSUM") as psp,
        ):
            # ---- persistent SBUF tiles ----
            wq_sb = wp.tile([128, 2 * D], BF16)
            wk_sb = wp.tile([128, 2 * D], BF16)
            wv_sb = wp.tile([128, 8 * FPC], BF16)
            wo_sb = wp.tile([128, 2 * D], BF16)
            masks_sb = wp.tile([128, 4 * 1024], BF16)
            xT = xp.tile([128, 8 * T], BF16)   # tb-major: tb*4096 + kc*512 + t
            qT_sb = qk.tile([128, 2 * T], BF16)       # pair hp at cols hp*T
            kT_sb = qk.tile([128, 2 * T], BF16)
            v_sb = vp.tile([128, NKT * HPC * VW], BF16)
            attT_sb = at.tile([128, 2 * T], BF16)
            scr_sb = wp.tile([128, 512], BF16)        # warmup matmul operands

            # warm the ACT exp table during the input DMAs
            warm_a = nr.tile([1, 8], F32, tag="warm", bufs=1)
            warm_b = nr.tile([1, 8], F32, tag="warm2", bufs=1)
            nc.vector.memset(warm_a[:], 0.0)
            nc.scalar.activation(warm_b[:], warm_a[:], mybir.ActivationFunctionType.Exp)
            nc.vector.memset(scr_sb[:], 0.0)
            # force the gpsimd IRAM library load early (off the critical path)
            warm_g = nr.tile([2, 8], F32, tag="warmg", bufs=1)
            nc.gpsimd.partition_broadcast(warm_g[:], warm_a[0:1, :], channels=2)
            # v slot layout: col 0 = ones (denominator lands on psum partition
            # 0 for partition_broadcast), cols 1:64 zero pad, cols 64:128 =
            # v dims (aligned partition base; 128-wide stationary for FWL)
            nc.gpsimd.memset(
                v_sb[:].rearrange("p (a b) -> p a b", b=VW)[:, :, 0], 1.0
            )
            nc.gpsimd.memset(
                v_sb[:].rearrange("p (a b) -> p a b", b=VW)[:, :, 1:64], 0.0
            )

            # ---- input DMAs, priority order ----
            nc.sync.dma_start(
                xT[:, 0:2048].rearrange("p (c t) -> p c t", t=512),
                xt_d.rearrange("(c p) t -> p c t", p=128)[:, 0:4, 0:512],
            )
            nc.sync.dma_start(wq_sb[:, 0:D], wq_d[:, 0:D])
            nc.sync.dma_start(
                xT[:, 2048:4096].rearrange("p (c t) -> p c t", t=512),
                xt_d.rearrange("(c p) t -> p c t", p=128)[:, 4:8, 0:512],
            )
            nc.sync.dma_start(wk_sb[:, 0:D], wk_d[:, 0:D])
            nc.sync.dma_start(wq_sb[:, D : 2 * D], wq_d[:, D : 2 * D])
            nc.sync.dma_start(wk_sb[:, D : 2 * D], wk_d[:, D : 2 * D])
            nc.sync.dma_start(masks_sb[:], masks_d)
            nc.sync.dma_start(wv_sb[:], wv_d)
            for tb in range(1, 4):
                nc.sync.dma_start(
                    xT[:, tb * 4096 : (tb + 1) * 4096].rearrange("p (c t) -> p c t", t=512),
                    xt_d.rearrange("(c p) t -> p c t", p=128)[:, :, tb * 512 : (tb + 1) * 512],
                )
            nc.sync.dma_start(wo_sb[:], wo_d)

            # ---- PE warmup: dummy matmuls on scratch while inputs stream ----
            wps = psp.tile([128, 512], F32, tag="proj", bufs=2, name="warm_ps")
            for i in range(N_WARM_MM):
                nc.tensor.matmul(
                    wps[:], scr_sb[:, 0:128], scr_sb[:],
                    start=(i == 0), stop=(i == N_WARM_MM - 1),
                )

            # ---- emission helpers ----
            emitted = set()
            backlog = deque()
            pend = deque()
            scalar_casts = [0]  # early proj casts routed to the idle ACT engine

            def proj_cast(dst, src):
                if scalar_casts[0] > 0:
                    scalar_casts[0] -= 1
                    nc.scalar.copy(dst, src)
                else:
                    nc.vector.tensor_copy(dst, src)

            def emit_qk_half(hp, tb, half):
                w_sb, dst = ((wq_sb, qT_sb), (wk_sb, kT_sb))[half]
                ps = psp.tile([128, 512], F32, tag="proj", bufs=2, name="qk_ps")
                for kc in range(8):
                    nc.tensor.matmul(
                        ps[:],
                        w_sb[:, hp * D + kc * 128 : hp * D + (kc + 1) * 128],
                        xT[:, tb * 4096 + kc * 512 : tb * 4096 + (kc + 1) * 512],
                        start=(kc == 0), stop=(kc == 7),
                    )
                proj_cast(dst[:, hp * T + tb * 512 : hp * T + (tb + 1) * 512], ps[:])

            def do_qk_half(hp, tb, half):
                key = ("qk", hp, tb, half)
                if key in emitted:
                    return
                emitted.add(key)
                emit_qk_half(hp, tb, half)

            def do_qk(hp, tb):
                do_qk_half(hp, tb, 0)
                do_qk_half(hp, tb, 1)

            def emit_v(tt):
                ps = psp.tile([128, 512], F32, tag="proj", bufs=2, name="v_ps")
                for kc in range(8):
                    nc.tensor.matmul(
                        ps[:, 0:FPC],
                        xT[:, (tt // 4) * 4096 + kc * 512 + (tt % 4) * 128 : (tt // 4) * 4096 + kc * 512 + (tt % 4) * 128 + 128],
                        wv_sb[:, kc * FPC : (kc + 1) * FPC],
                        start=(kc == 0), stop=(kc == 7),
                    )
                proj_cast(
                    v_sb[:].rearrange("p (a b) -> p a b", b=VW)[
                        :, tt * HPC : (tt + 1) * HPC, 64:128
                    ],
                    ps[:, 0:FPC].rearrange("p (a b) -> p a b", b=DH),
                )

            def do_v(tt):
                key = ("v", tt)
                if key in emitted:
                    return
                emitted.add(key)
                emit_v(tt)

            o_sb_map = {}
            op_stage = {}
            pre_ps = {}

            def emit_outproj_half(qb, t4, nck):
                tt = qb * 4 + t4
                if nck == 0:
                    o_sb_map[tt] = op.tile([128, D], BF16, tag="osb", name="o_sb")
                o_sb = o_sb_map[tt]
                if (tt, nck) in pre_ps:
                    # hp0 half was pre-accumulated during the final norm chain
                    wo_ps = pre_ps.pop((tt, nck))
                    hps = (1,)
                elif qb == 3 and (t4 * 2 + nck) % 2 == 1:
                    # tail: the score banks are dead, borrow them
                    wo_ps = pss.tile([128, 512], F32, tag="sAB", bufs=2, name="wo_ps2")
                    hps = (0, 1)
                else:
                    wo_ps = psp.tile([128, 512], F32, tag="proj", bufs=2, name="wo_ps")
                    hps = (0, 1)
                for hp in hps:
                    nc.tensor.matmul(
                        wo_ps[:],
                        attT_sb[:, hp * T + tt * 128 : hp * T + (tt + 1) * 128],
                        wo_sb[:, hp * D + nck * 512 : hp * D + (nck + 1) * 512],
                        start=(hp == 0), stop=(hp == 1),
                    )
                if qb == 3 and nck == 0:
                    # tail: exp stream is done, use the idle scalar engine
                    nc.scalar.copy(o_sb[:, 0:512], wo_ps[:])
                else:
                    nc.vector.tensor_copy(o_sb[:, nck * 512 : (nck + 1) * 512], wo_ps[:])
                if qb == 3:
                    # tail: per-half DMAs on alternating idle queues
                    eng = nc.sync if (tt * 2 + nck) % 2 == 1 else nc.gpsimd
                    eng.dma_start(
                        out_d[tt * 128 : (tt + 1) * 128, nck * 512 : (nck + 1) * 512],
                        o_sb[:, nck * 512 : (nck + 1) * 512],
                    )
                elif nck == 1:
                    nc.gpsimd.dma_start(out_d[tt * 128 : (tt + 1) * 128, :], o_sb[:])

            def emit_scores(qb, hp, kt):
                sAB = pss.tile([128, 1024], F32, tag="sAB", bufs=2, name="sAB")
                r = kt - 4 * qb
                qs = 128 * r if (r > 0 and DIAG_SKIP) else 0
                for h, tp in ((0, (0, 0)), (1, (64, 0))):
                    nc.tensor.matmul(
                        sAB[:, h * 512 + qs : (h + 1) * 512],
                        kT_sb[h * 64 : (h + 1) * 64, hp * T + kt * 128 : hp * T + (kt + 1) * 128],
                        qT_sb[h * 64 : (h + 1) * 64, hp * T + qb * 512 + qs : hp * T + (qb + 1) * 512],
                        start=True, stop=True, tile_position=tp,
                    )
                return sAB, qs

            def emit_act_mask(qb, hp, kt, sAB, qs):
                r = kt - 4 * qb
                eAB = ep.tile([128, 1024], BF16, tag="eAB", bufs=12, name="eAB")
                if qs == 0:
                    nc.scalar.activation(
                        eAB[:], sAB[:], mybir.ActivationFunctionType.Exp, scale=0.125
                    )
                    if r >= 0:
                        nc.vector.tensor_mul(
                            eAB[:], eAB[:], masks_sb[:, r * 1024 : (r + 1) * 1024]
                        )
                else:
                    iv = sAB[:].rearrange("p (h q) -> p h q", h=2)[:, :, qs:512]
                    ov = eAB[:].rearrange("p (h q) -> p h q", h=2)[:, :, qs:512]
                    nc.scalar.activation(
                        ov, iv, mybir.ActivationFunctionType.Exp, scale=0.125
                    )
                    mv = masks_sb[:, r * 1024 : (r + 1) * 1024].rearrange(
                        "p (h q) -> p h q", h=2
                    )[:, :, qs:512]
                    nc.vector.tensor_mul(ov, ov, mv)
                return eAB

            def emit_attv(qb, hp, kt, eAB, qs, oA, oB):
                nkt = 4 * (qb + 1)
                for h, o_ps in ((0, oA), (1, oB)):
                    nc.tensor.matmul(
                        o_ps[:, qs:512],
                        v_sb[:, (kt * HPC + 2 * hp + h) * VW : (kt * HPC + 2 * hp + h + 1) * VW],
                        eAB[:, h * 512 + qs : (h + 1) * 512],
                        start=(kt == 0), stop=(kt == nkt - 1),
                    )

            def emit_norm(ui, qb, hp, oA, oB):
                oAs = ob.tile([128, 512], F32, tag="oAs", bufs=2, name="oAs")
                oBs = ob.tile([128, 512], F32, tag="oBs", bufs=2, name="oBs")
                nc.vector.tensor_copy(oAs[:], oA[:])
                nc.vector.tensor_copy(oBs[:], oB[:])
                packed = nr.tile([128, 8], F32, tag="packed", name="packed")
                nc.sync.dma_start(
                    packed[:, 0:4], oAs[0:1, :].rearrange("r (g e) -> r g e", e=4)
                )
                nc.sync.dma_start(
                    packed[:, 4:8], oBs[0:1, :].rearrange("r (g e) -> r g e", e=4)
                )
                rpacked = nr.tile([128, 8], F32, tag="rpacked", name="rpacked")
                nc.vector.reciprocal(rpacked[:], packed[:])
                rrecs = []
                for h in range(2):
                    rrec = nr.tile([1, 512], F32, tag=f"rrec{h}", name="rrec")
                    nc.sync.dma_start(
                        rrec[:].rearrange("r (g e) -> r g e", e=4),
                        rpacked[:, 4 * h : 4 * h + 4],
                    )
                    rrecs.append(rrec)
                bcs = []
                for rrec in rrecs:
                    bc = nr.tile([128, 512], F32, tag="bc", bufs=4, name="bc")
                    nc.gpsimd.partition_broadcast(bc[:], rrec[:], channels=128)
                    bcs.append(bc)

                def muls(oAs=oAs, oBs=oBs, bcs=bcs, qb=qb, hp=hp):
                    for o_s, bc, prow in ((oAs, bcs[0], 0), (oBs, bcs[1], 64)):
                        nc.vector.tensor_mul(
                            attT_sb[prow : prow + 64, hp * T + qb * 512 : hp * T + (qb + 1) * 512],
                            o_s[64:128, :],
                            bc[64:128, :],
                        )
                backlog.append(muls)
                if hp == 1:
                    items = [
                        (lambda qb=qb, t4=t4, nck=nck: emit_outproj_half(qb, t4, nck))
                        for t4 in range(4)
                        for nck in range(2)
                    ]
                    if qb <= 1:
                        op_stage[qb] = items  # keep out of the early PE crunch
                    else:
                        backlog.extend(items)
                    if qb == 3:
                        # pre-run the hp0 contraction half for the first 2
                        # token tiles while the norm chain latency plays out
                        for t4 in range(2):
                            for nck in range(2):
                                tt = 12 + t4
                                wo_ps = (
                                    psp.tile([128, 512], F32, tag="proj", bufs=2, name="wo_pre")
                                    if nck == 0
                                    else pss.tile([128, 512], F32, tag="sAB", bufs=2, name="wo_pre2")
                                )
                                nc.tensor.matmul(
                                    wo_ps[:],
                                    attT_sb[:, tt * 128 : (tt + 1) * 128],
                                    wo_sb[:, nck * 512 : (nck + 1) * 512],
                                    start=True, stop=False,
                                )
                                pre_ps[(tt, nck)] = wo_ps

            def pop_attv():
                batch = [pend.popleft()]
                if pend and pend[0][3] == batch[0][3]:
                    batch.append(pend.popleft())
                for e in batch:
                    do_v(e[2])
                for e in list(pend)[:1]:
                    do_v(e[2])  # prefetch next pop's v so attv won't wait the cast
                for h in (0, 1):
                    for qb_, hp_, kt_, ui_, eAB_, qs_, oA_, oB_ in batch:
                        o_ps = oA_ if h == 0 else oB_
                        nc.tensor.matmul(
                            o_ps[:, qs_:512],
                            v_sb[:, (kt_ * HPC + 2 * hp_ + h) * VW : (kt_ * HPC + 2 * hp_ + h + 1) * VW],
                            eAB_[:, h * 512 + qs_ : (h + 1) * 512],
                            start=(kt_ == 0), stop=(kt_ == 4 * (qb_ + 1) - 1),
                        )
                for qb_, hp_, kt_, ui_, eAB_, qs_, oA_, oB_ in batch:
                    if kt_ == 4 * (qb_ + 1) - 1:
                        emit_norm(ui_, qb_, hp_, oA_, oB_)

            def drip(n):
                for _ in range(n):
                    if backlog:
                        backlog.popleft()()

            def qk_items(hp, tb):
                return [
                    (lambda hp=hp, tb=tb: do_qk_half(hp, tb, 0)),
                    (lambda hp=hp, tb=tb: do_qk_half(hp, tb, 1)),
                ]

            unit_pushes = {
                0: qk_items(1, 0),
                1: qk_items(0, 1),
                2: qk_items(1, 1) + [
                    (lambda tt=tt: do_v(tt)) for tt in range(4, 8)
                ],
                3: qk_items(0, 2) + [
                    (lambda tt=tt: do_v(tt)) for tt in range(8, 12)
                ],
                4: qk_items(1, 2),
                5: qk_items(0, 3) + [
                    (lambda tt=tt: do_v(tt)) for tt in range(12, 16)
                ],
                6: qk_items(1, 3),
                7: [],
            }

            # ---- main pipeline ----
            do_qk(0, 0)
            ui = 0
            for qb in range(NQB):
                for hp in range(2):
                    do_qk(hp, qb)
                    backlog.extend(unit_pushes[ui])
                    if qb >= 2:
                        backlog.extend(op_stage.pop(qb - 2, []))
                    oA = pso.tile([128, 512], F32, tag="oA", bufs=1, name="oA")
                    oB = pso.tile([128, 512], F32, tag="oB", bufs=1, name="oB")
                    nkt = 4 * (qb + 1)
                    # defer qb0 attvs (pend depth 6) so the tiny early units
                    # spend PE time on scores+projections only; drain the
                    # backlog gradually during (1,0)
                    for kt in range(nkt):
                        sAB, qs = emit_scores(qb, hp, kt)
                        eAB = emit_act_mask(qb, hp, kt, sAB, qs)
                        if qb > 0:
                            do_v(kt)  # ensure v ready 2 steps before its attv
                        pend.append((qb, hp, kt, ui, eAB, qs, oA, oB))
                        if qb == 0:
                            depth = 8
                        elif qb == 1 and hp == 0:
                            depth = max(3, 8 - kt)
                        else:
                            depth = 3
                        while len(pend) > depth:
                            pop_attv()
                        # no drip on the unit's last step (it would delay the
                        # next unit's first scores) nor on (1,0)'s first two
                        # steps (the deferred qb0 drain + v chains peak there)
                        if kt < nkt - 1 and not (qb == 1 and hp == 0 and kt < 2):
                            drip(1)
                    ui += 1
            while pend:
                pop_attv()
            tw = pso.tile([128, 512], F32, tag="oA", bufs=1, name="tail_warm")
            for i in range(18):
                nc.tensor.matmul(
                    tw[:], scr_sb[:, 0:128], scr_sb[:],
                    start=(i == 0), stop=(i == 17),
                )
            drip(len(backlog))

    nc.compile()
    return nc


def _prepack(w, bf):
    # [c*128, f] -> [128, c*f] (SBUF chunk layout)
    c = w.shape[0] // 128
    return np.ascontiguousarray(
        w.reshape(c, 128, w.shape[1]).transpose(1, 0, 2).reshape(128, -1)
    ).astype(bf)


def _prepack_pair(w, bf):
    # w: [1024, 256] (d_model x pair features) -> [128, 2*1024] pair-major:
    # wp[row, p*1024 + kc*128 + f] = w[kc*128+row, p*128+f]
    t = np.asarray(w).reshape(8, 128, 2, 128).transpose(1, 2, 0, 3).reshape(128, 2048)
    return np.ascontiguousarray(t).astype(bf)


def _prep_in_maps(x, Wq, Wk, Wv, Wo):
    x = np.asarray(x, dtype=np.float32)
    bf = ml_dtypes.bfloat16
    Wq = np.asarray(Wq, dtype=np.float32)
    Wk = np.asarray(Wk, dtype=np.float32)
    Wv = np.asarray(Wv, dtype=np.float32)
    Wo = np.asarray(Wo, dtype=np.float32)
    ii = np.arange(128)[:, None]
    qq = np.arange(512)[None, :]
    masks = np.concatenate(
        [np.tile((qq >= ii + 128 * r).astype(bf), (1, 2)) for r in range(4)],
        axis=1,
    )
    in_maps = []
    for c in range(8):
        b, g = divmod(c, 4)
        sl = slice(g * FPC, (g + 1) * FPC)
        in_maps.append(
            {
                "xt": np.ascontiguousarray(x[b].T).astype(bf),
                "wq_p": _prepack_pair(Wq[sl, :].T, bf),
                "wk_p": _prepack_pair(Wk[sl, :].T, bf),
                "wv_t": _prepack(Wv[sl, :].T, bf),
                "wo_t": _prepack(Wo[:, sl].T, bf),
                "masks": masks,
            }
        )
    return in_maps


def _get_nc():
    if "nc" not in _CACHE:
        _CACHE["nc"] = _build()
    return _CACHE["nc"]


def _assemble(results):
    out = np.empty((B, T, D), dtype=np.float32)
    for b in range(B):
        out[b] = (
            results[4 * b]["po"].astype(np.float32)
            + results[4 * b + 1]["po"].astype(np.float32)
            + results[4 * b + 2]["po"].astype(np.float32)
            + results[4 * b + 3]["po"].astype(np.float32)
        )
    return out


def kernel(x, Wq, Wk, Wv, Wo):
    nc = _get_nc()
    in_maps = _prep_in_maps(x, Wq, Wk, Wv, Wo)
    res = run_bass_kernel_spmd(nc, in_maps, core_ids=list(range(8)))
    return _assemble(res.results)


def kernel_with_trace(x, Wq, Wk, Wv, Wo, **kw):
    nc = _get_nc()
    in_maps = _prep_in_maps(x, Wq, Wk, Wv, Wo)
    res = run_bass_kernel_spmd(nc, in_maps, core_ids=list(range(8)), trace=True, **kw)
    return _assemble(res.results), res
